# revision 2
# baseline (speedup 1.0000x reference)
"""AudioOnlyOnTheFlyModel kernel.

reference: y = (chirp * rir)[:2646] (full linear convolution per batch &
channel), then a torchaudio magnitude spectrogram (n_fft=512, hann
win=64, hop=16, center=True reflect pad) -> output (64, 2, 257, 166) f32.

Exact algebraic restructuring (all single-core CPU, f32):
  1. The first 2646 conv samples depend only on the first 2646 samples
     of chirp and rir; an alias-free circular conv needs N >= 5291, so
     use f32 FFTs at N = 5400 = 2^3*3^3*5^2 (pocketfft-fast) instead of
     the reference's 131072-point f64 FFT.
  2. The center-padded Hann window has 64 nonzero taps, so each STFT
     frame is a 64-tap windowed DFT. Two exact symmetry folds shrink
     that GEMM 4x relative to the naive (64 x 514) windowed-DFT matrix:
       - Hann tap symmetry  A[64-j] =  A[j],        B[64-j] = -B[j]
       - frequency fold     A[j,256-f] = (-1)^j A[j,f], B likewise -
     leaving 4 batched GEMMs with K = 16/15 over 129 frequencies.
  3. GEMMs and the butterfly+magnitude pass are chunked over the batch
     so intermediates stay cache-resident; the magnitude pass writes
     the final (64, 2, 257, 166) layout directly (no transposes).

torch supplies AVX-512 single-core GEMM/FFT; numba compiles the fused
gather/butterfly passes. A pure-numpy fallback covers their absence.
"""
import numpy as np

USEFUL = 2646
NFFT = 512
WIN = 64
HOP = 16
BATCH = 64
NF = 1 + USEFUL // HOP          # 166 frames
NBIN = NFFT // 2 + 1            # 257 bins
CFFT = 5400
BC = BATCH * 2                  # 128
NFRQ = 129                      # folded frequencies (0..128)
G = 2                           # bc's per GEMM batch column-group
NB = BC // G                    # 64
NG = G * NF                     # 332
CH = 8                          # groups per GEMM/bmag chunk (16 bc)

try:
    import torch
    import numba
    _FAST = True
except Exception:               # pragma: no cover - defensive fallback
    _FAST = False


def _win_dft(nfrq):
    n = np.arange(WIN, dtype=np.float64)
    w = 0.5 * (1.0 - np.cos(2.0 * np.pi * n / WIN))
    j = np.arange(WIN, dtype=np.float64)[:, None]
    f = np.arange(nfrq, dtype=np.float64)[None, :]
    ph = 2.0 * np.pi * f * (224.0 + j) / NFFT
    return w[:, None] * np.cos(ph), w[:, None] * np.sin(ph)


if _FAST:
    import os
    try:
        _NT = max(1, min(8, len(os.sched_getaffinity(0))))
    except Exception:
        _NT = 1
    torch.set_num_threads(_NT)

    def _dft_mats():
        A, B = _win_dft(NFRQ)
        jev = list(range(2, 32, 2))
        jod = list(range(1, 32, 2))
        Aev = np.concatenate([A[jev], A[32:33]], axis=0)   # (16, 129)
        c = np.ascontiguousarray
        return (c(Aev.T, np.float32), c(A[jod].T, np.float32),
                c(B[jod].T, np.float32), c(B[jev].T, np.float32))

    _AevT, _AodT, _BodT, _BevT = (torch.from_numpy(m) for m in _dft_mats())

    _RU = np.empty((BATCH, 2, USEFUL), np.float32)
    _CU = np.empty((2, USEFUL), np.float32)
    _Eev = np.empty((NB, 16, NG), np.float32)
    _Eod = np.empty((NB, 16, NG), np.float32)
    _Ood = np.empty((NB, 16, NG), np.float32)
    _Oev = np.empty((NB, 15, NG), np.float32)
    _Eev_t, _Eod_t, _Ood_t, _Oev_t = (torch.from_numpy(a) for a in
                                      (_Eev, _Eod, _Ood, _Oev))
    _Pe = torch.empty((CH, NFRQ, NG), dtype=torch.float32)
    _Po = torch.empty((CH, NFRQ, NG), dtype=torch.float32)
    _Qo = torch.empty((CH, NFRQ, NG), dtype=torch.float32)
    _Qe = torch.empty((CH, NFRQ, NG), dtype=torch.float32)
    _Pe_np, _Po_np, _Qo_np, _Qe_np = (t.numpy() for t in
                                      (_Pe, _Po, _Qo, _Qe))
    _OUT = np.empty((BATCH, 2, NBIN, NF), np.float32)
    _OUT2 = _OUT.reshape(BC, NBIN, NF)
    _TMP = np.empty((NFRQ, NG), np.float32)

    @numba.njit(fastmath=True)
    def _eo4(y, Eev, Eod, Ood, Oev):
        # windowed-frame gather + even/odd tap fold, straight from the
        # conv output (reflect handling only at frames t in {0,1,164,165})
        for bc in range(128):
            row = y[bc]
            g = bc >> 1
            off = (bc & 1) * 166
            for ji in range(15):
                j = 2 * ji + 2
                jm = j - 32
                km = 32 - j
                for t in range(2, 164):
                    a = row[16 * t + jm]
                    b = row[16 * t + km]
                    Eev[g, ji, off + t] = a + b
                    Oev[g, ji, off + t] = a - b
                for t in (0, 1, 164, 165):
                    na = 16 * t + jm
                    if na < 0:
                        na = -na
                    nb = 16 * t + km
                    if nb >= 2646:
                        nb = 5290 - nb
                    a = row[na]
                    b = row[nb]
                    Eev[g, ji, off + t] = a + b
                    Oev[g, ji, off + t] = a - b
            for t in range(166):
                Eev[g, 15, off + t] = row[16 * t]
            for ji in range(16):
                j = 2 * ji + 1
                jm = j - 32
                km = 32 - j
                for t in range(2, 164):
                    a = row[16 * t + jm]
                    b = row[16 * t + km]
                    Eod[g, ji, off + t] = a + b
                    Ood[g, ji, off + t] = a - b
                for t in (0, 1, 164, 165):
                    na = 16 * t + jm
                    if na < 0:
                        na = -na
                    nb = 16 * t + km
                    if nb >= 2646:
                        nb = 5290 - nb
                    a = row[na]
                    b = row[nb]
                    Eod[g, ji, off + t] = a + b
                    Ood[g, ji, off + t] = a - b

    @numba.njit(fastmath=True)
    def _bmag(Pe, Po, Qo, Qe, out, tmp, g0):
        # out[bc, f]     = |(Pe+Po) + i(Qo+Qe)|   f = 0..128
        # out[bc, 256-f] = |(Pe-Po) + i(Qo-Qe)|   f = 0..127
        # hi rows staged in tmp (keeps stores alias-free & vectorized)
        for gi in range(Pe.shape[0]):
            g = g0 + gi
            pe = Pe[gi]; po = Po[gi]; qo = Qo[gi]; qe = Qe[gi]
            for f in range(129):
                pef = pe[f]; pof = po[f]; qof = qo[f]; qef = qe[f]
                o0 = out[2 * g, f]
                o1 = out[2 * g + 1, f]
                th = tmp[f]
                for t in range(166):
                    a = pef[t]; b = pof[t]; c = qof[t]; d = qef[t]
                    lr = a + b; li = c + d
                    hr = a - b; hi = c - d
                    o0[t] = np.sqrt(lr * lr + li * li)
                    th[t] = np.sqrt(hr * hr + hi * hi)
                for t in range(166):
                    a = pef[166 + t]; b = pof[166 + t]
                    c = qof[166 + t]; d = qef[166 + t]
                    lr = a + b; li = c + d
                    hr = a - b; hi = c - d
                    o1[t] = np.sqrt(lr * lr + li * li)
                    th[166 + t] = np.sqrt(hr * hr + hi * hi)
            for f in range(128):
                orow0 = out[2 * g, 256 - f]
                orow1 = out[2 * g + 1, 256 - f]
                trow = tmp[f]
                for t in range(166):
                    orow0[t] = trow[t]
                    orow1[t] = trow[166 + t]

    def kernel(rir, chirp):
        rir = np.asarray(rir, dtype=np.float32)
        chirp = np.asarray(chirp, dtype=np.float32)

        np.copyto(_RU, rir[:, :, :USEFUL])
        np.copyto(_CU, chirp[:, :USEFUL])
        Rf = torch.fft.rfft(torch.from_numpy(_RU), CFFT)
        Cf = torch.fft.rfft(torch.from_numpy(_CU), CFFT)
        Rf.mul_(Cf)
        y = torch.fft.irfft(Rf, CFFT).numpy().reshape(BC, CFFT)

        _eo4(y, _Eev, _Eod, _Ood, _Oev)
        for i in range(0, NB, CH):
            sl = slice(i, i + CH)
            torch.matmul(_AevT, _Eev_t[sl], out=_Pe)
            torch.matmul(_AodT, _Eod_t[sl], out=_Po)
            torch.matmul(_BodT, _Ood_t[sl], out=_Qo)
            torch.matmul(_BevT, _Oev_t[sl], out=_Qe)
            _bmag(_Pe_np, _Po_np, _Qo_np, _Qe_np, _OUT2, _TMP, i)
        return _OUT

    def _warm():
        rir = np.zeros((BATCH, 2, 44100), np.float32)
        chirp = np.zeros((2, 44100), np.float32)
        for _ in range(6):
            kernel(rir, chirp)

    _warm()

else:
    # numpy-only fallback: same algebra, unfolded 64x514 windowed DFT
    def _ab():
        A, B = _win_dft(NBIN)
        return np.concatenate([A, B], axis=1).astype(np.float32)

    _AB = _ab()

    def kernel(rir, chirp):
        rir = np.asarray(rir, dtype=np.float32)
        chirp = np.asarray(chirp, dtype=np.float32)
        ru = rir[..., :USEFUL]
        cu = chirp[..., :USEFUL]
        Cf = np.fft.rfft(cu, CFFT)
        Rf = np.fft.rfft(ru, CFFT)
        y = np.fft.irfft(Cf[None] * Rf, CFFT)[..., :USEFUL]
        y = np.ascontiguousarray(y).astype(np.float32)
        yp = np.pad(y, ((0, 0), (0, 0), (32, 32)), mode="reflect")
        s = yp.strides
        Y = np.lib.stride_tricks.as_strided(
            yp, shape=(BATCH, 2, NF, WIN),
            strides=(s[0], s[1], s[2] * HOP, s[2]))
        Yf = np.ascontiguousarray(Y).reshape(-1, WIN)
        out = Yf @ _AB
        re = out[:, :NBIN]
        im = out[:, NBIN:]
        np.multiply(re, re, out=re)
        np.multiply(im, im, out=im)
        re += im
        result = np.empty((BATCH, 2, NBIN, NF), np.float32)
        np.sqrt(re.reshape(BATCH, 2, NF, NBIN).swapaxes(-1, -2),
                out=result)
        return result


# revision 4
# speedup vs baseline: 1.2916x; 1.2916x over previous
"""AudioOnlyOnTheFlyModel kernel.

reference: y = (chirp * rir)[:2646] (full linear convolution per batch &
channel), then a torchaudio magnitude spectrogram (n_fft=512, hann
win=64, hop=16, center=True reflect pad) -> output (64, 2, 257, 166) f32.

Exact algebraic restructuring (single-core CPU, f32):
  1. The first 2646 conv samples depend only on the first 2646 samples
     of chirp and rir; an alias-free circular conv needs N >= 5291, so
     f32 FFTs at N = 5400 = 2^3*3^3*5^2 replace the reference's
     131072-point f64 FFTs.
  2. The center-padded Hann window has 64 nonzero taps, so each STFT
     frame is a 64-tap windowed DFT. Two exact symmetry folds shrink
     that GEMM 4x vs the naive (64 x 514) windowed-DFT matrix:
       - Hann tap symmetry  A[64-j] =  A[j],  B[64-j] = -B[j]
       - frequency fold     A[j,256-f] = (-1)^j A[j,f], B likewise
     leaving four K=16 tap-folded operands, multiplied in ONE torch.bmm
     against a pre-stacked (4*CH,129,16) matrix block per chunk.
  3. The whole pipeline (FFT conv -> tap fold -> bmm -> butterfly +
     magnitude) runs in 8 batch-chunks so intermediates stay
     cache-resident; the magnitude pass writes the final
     (64, 2, 257, 166) layout directly (no transposes). Big buffers are
     madvise(HUGEPAGE|COLLAPSE)'d to cut TLB refills on cold calls.

torch supplies AVX-512 single-core GEMM/FFT; numba compiles the fused
gather/butterfly passes. A pure-numpy fallback covers their absence.
"""
import numpy as np

USEFUL = 2646
NFFT = 512
WIN = 64
HOP = 16
BATCH = 64
NF = 1 + USEFUL // HOP          # 166 frames
NBIN = NFFT // 2 + 1            # 257 bins
CFFT = 5400
BC = BATCH * 2                  # 128
NFRQ = 129                      # folded frequencies (0..128)
G = 2                           # bc's per GEMM batch column-group
NB = BC // G                    # 64
NG = G * NF                     # 332
CH = 8                          # groups per chunk (16 bc)
NCHUNK = NB // CH               # 8
BCH = 8                         # batch items per chunk

try:
    import torch
    import numba
    _FAST = True
except Exception:               # pragma: no cover - defensive fallback
    _FAST = False


def _win_dft(nfrq):
    n = np.arange(WIN, dtype=np.float64)
    w = 0.5 * (1.0 - np.cos(2.0 * np.pi * n / WIN))
    j = np.arange(WIN, dtype=np.float64)[:, None]
    f = np.arange(nfrq, dtype=np.float64)[None, :]
    ph = 2.0 * np.pi * f * (224.0 + j) / NFFT
    return w[:, None] * np.cos(ph), w[:, None] * np.sin(ph)


if _FAST:
    import os
    import ctypes
    try:
        _NT = max(1, min(8, len(os.sched_getaffinity(0))))
    except Exception:
        _NT = 1
    torch.set_num_threads(_NT)

    def _lhs_stack():
        A, B = _win_dft(NFRQ)
        jev = list(range(2, 32, 2))
        jod = list(range(1, 32, 2))
        Aev = np.concatenate([A[jev], A[32:33]], axis=0)       # (16, 129)
        Aod = A[jod]
        Bod = B[jod]
        Bev = np.concatenate([B[jev], np.zeros((1, NFRQ))], 0)  # K pad
        mats = [Aev.T, Aod.T, Bod.T, Bev.T]                     # (129, 16)
        L = np.stack([np.ascontiguousarray(m, np.float32) for m in mats])
        Lbig = np.repeat(L[:, None], CH, axis=1).reshape(4 * CH, NFRQ, 16)
        return torch.from_numpy(np.ascontiguousarray(Lbig))

    _LHS = _lhs_stack()

    _RU = np.empty((BATCH, 2, USEFUL), np.float32)
    _RU_t = torch.from_numpy(_RU)
    _CU = np.empty((2, USEFUL), np.float32)
    _E4 = np.zeros((4, CH, 16, NG), np.float32)   # zeroed: pad row stays 0
    _E4_t = torch.from_numpy(_E4).view(4 * CH, 16, NG)
    _P4 = torch.empty((4 * CH, NFRQ, NG), dtype=torch.float32)
    _P4_np = _P4.numpy()
    _OUT = np.empty((BATCH, 2, NBIN, NF), np.float32)
    _OUT2 = _OUT.reshape(BC, NBIN, NF)
    _TMP = np.empty((NFRQ, NG), np.float32)

    def _madvise_huge(*arrays):
        try:
            libc = ctypes.CDLL(None, use_errno=True)
            page = 4096
            for a in arrays:
                addr = a.ctypes.data
                start = addr & ~(page - 1)
                length = (addr + a.nbytes) - start
                libc.madvise(ctypes.c_void_p(start),
                             ctypes.c_size_t(length), 14)   # MADV_HUGEPAGE
                libc.madvise(ctypes.c_void_p(start),
                             ctypes.c_size_t(length), 25)   # MADV_COLLAPSE
        except Exception:
            pass

    @numba.njit(fastmath=True)
    def _eo4(y, E4):
        # windowed-frame tap gather + even/odd fold straight from the
        # conv chunk; reflect handling only at frames t in {0,1,164,165}.
        # E4 quarters: rhs for Aev / Aod / Bod / Bev(zero-padded K row).
        for bcl in range(16):
            row = y[bcl]
            g = bcl >> 1
            off = (bcl & 1) * 166
            for ji in range(15):
                j = 2 * ji + 2
                jm = j - 32
                km = 32 - j
                for t in range(2, 164):
                    a = row[16 * t + jm]
                    b = row[16 * t + km]
                    E4[0, g, ji, off + t] = a + b
                    E4[3, g, ji, off + t] = a - b
                for t in (0, 1, 164, 165):
                    na = 16 * t + jm
                    if na < 0:
                        na = -na
                    nb = 16 * t + km
                    if nb >= 2646:
                        nb = 5290 - nb
                    a = row[na]
                    b = row[nb]
                    E4[0, g, ji, off + t] = a + b
                    E4[3, g, ji, off + t] = a - b
            for t in range(166):
                E4[0, g, 15, off + t] = row[16 * t]
            for ji in range(16):
                j = 2 * ji + 1
                jm = j - 32
                km = 32 - j
                for t in range(2, 164):
                    a = row[16 * t + jm]
                    b = row[16 * t + km]
                    E4[1, g, ji, off + t] = a + b
                    E4[2, g, ji, off + t] = a - b
                for t in (0, 1, 164, 165):
                    na = 16 * t + jm
                    if na < 0:
                        na = -na
                    nb = 16 * t + km
                    if nb >= 2646:
                        nb = 5290 - nb
                    a = row[na]
                    b = row[nb]
                    E4[1, g, ji, off + t] = a + b
                    E4[2, g, ji, off + t] = a - b

    @numba.njit(fastmath=True)
    def _bmag(P4, out, tmp, g0):
        # out[bc, f]     = |(Pe+Po) + i(Qo+Qe)|   f = 0..128
        # out[bc, 256-f] = |(Pe-Po) + i(Qo-Qe)|   f = 0..127
        # hi rows staged in tmp (keeps stores alias-free & vectorized)
        for gi in range(8):
            g = g0 + gi
            pe = P4[gi]; po = P4[8 + gi]
            qo = P4[16 + gi]; qe = P4[24 + gi]
            for f in range(129):
                pef = pe[f]; pof = po[f]; qof = qo[f]; qef = qe[f]
                o0 = out[2 * g, f]
                o1 = out[2 * g + 1, f]
                th = tmp[f]
                for t in range(166):
                    a = pef[t]; b = pof[t]; c = qof[t]; d = qef[t]
                    lr = a + b; li = c + d
                    hr = a - b; hi = c - d
                    o0[t] = np.sqrt(lr * lr + li * li)
                    th[t] = np.sqrt(hr * hr + hi * hi)
                for t in range(166):
                    a = pef[166 + t]; b = pof[166 + t]
                    c = qof[166 + t]; d = qef[166 + t]
                    lr = a + b; li = c + d
                    hr = a - b; hi = c - d
                    o1[t] = np.sqrt(lr * lr + li * li)
                    th[166 + t] = np.sqrt(hr * hr + hi * hi)
            for f in range(128):
                orow0 = out[2 * g, 256 - f]
                orow1 = out[2 * g + 1, 256 - f]
                trow = tmp[f]
                for t in range(166):
                    orow0[t] = trow[t]
                    orow1[t] = trow[166 + t]

    def kernel(rir, chirp):
        rir = np.asarray(rir, dtype=np.float32)
        chirp = np.asarray(chirp, dtype=np.float32)

        np.copyto(_RU, rir[:, :, :USEFUL])
        np.copyto(_CU, chirp[:, :USEFUL])
        Cf = torch.fft.rfft(torch.from_numpy(_CU), CFFT)     # (2, 2701)

        for ci in range(NCHUNK):
            b0 = ci * BCH
            Rf = torch.fft.rfft(_RU_t[b0:b0 + BCH], CFFT)    # (8, 2, 2701)
            Rf.mul_(Cf)
            y = torch.fft.irfft(Rf, CFFT).numpy().reshape(16, CFFT)
            _eo4(y, _E4)
            torch.bmm(_LHS, _E4_t, out=_P4)
            _bmag(_P4_np, _OUT2, _TMP, ci * CH)
        return _OUT

    def _warm():
        rir = np.zeros((BATCH, 2, 44100), np.float32)
        chirp = np.zeros((2, 44100), np.float32)
        for _ in range(6):
            kernel(rir, chirp)

    _madvise_huge(_RU, _E4, _P4_np, _OUT)
    _warm()

else:
    # numpy-only fallback: same algebra, unfolded 64x514 windowed DFT
    def _ab():
        A, B = _win_dft(NBIN)
        return np.concatenate([A, B], axis=1).astype(np.float32)

    _AB = _ab()

    def kernel(rir, chirp):
        rir = np.asarray(rir, dtype=np.float32)
        chirp = np.asarray(chirp, dtype=np.float32)
        ru = rir[..., :USEFUL]
        cu = chirp[..., :USEFUL]
        Cf = np.fft.rfft(cu, CFFT)
        Rf = np.fft.rfft(ru, CFFT)
        y = np.fft.irfft(Cf[None] * Rf, CFFT)[..., :USEFUL]
        y = np.ascontiguousarray(y).astype(np.float32)
        yp = np.pad(y, ((0, 0), (0, 0), (32, 32)), mode="reflect")
        s = yp.strides
        Y = np.lib.stride_tricks.as_strided(
            yp, shape=(BATCH, 2, NF, WIN),
            strides=(s[0], s[1], s[2] * HOP, s[2]))
        Yf = np.ascontiguousarray(Y).reshape(-1, WIN)
        out = Yf @ _AB
        re = out[:, :NBIN]
        im = out[:, NBIN:]
        np.multiply(re, re, out=re)
        np.multiply(im, im, out=im)
        re += im
        result = np.empty((BATCH, 2, NBIN, NF), np.float32)
        np.sqrt(re.reshape(BATCH, 2, NF, NBIN).swapaxes(-1, -2),
                out=result)
        return result
